# revision 19
# baseline (speedup 1.0000x reference)
"""Trainium2 Bass kernel for AttentionBlock (B=8, C=256, L=2048), data-parallel
over batch across 8 NeuronCores.

Math (one batch per core, x: [C, L]):
    t^T   = w8^T x8            w8 = fp8(kappa M x),  M = Wq^T Wk,  kappa = 8*SCALE/ln2
    pT    = exp-ish(t)         [m, l], m on partitions; global shift cancels in softmax
    denom = ones^T acc(pT)     (two running bf16 accumulators, DVE + Pool)
    ctx   = vT8^T pT           vT8 = fp8(x^T Wv^T); ux (per-key bq.Wk x) rides along as
                               a 257th output column of the same projection
    out   = ctx * (1/denom) + (bf16(x) + bv)

All heavy matmuls run in fp8e4 with perf_mode=DoubleRow: operands are packed
[128, 2, free] so one instruction contracts 256 deep (2 k-tiles), ~1.5x bf16
throughput at FD>=512.

exp is split across two engines:
  - ACT chunks: nc.scalar.activation(Exp, scale=ln2/8, bias=ux-shift) -> fp8 direct
  - DVE chunks: Schraudolph-in-fp8: bits = clamp(t + b_dve, 0) as uint8, where
    b_dve = (8/ln2)(ux-shift) + 56; the uint8 bit pattern IS the fp8 exp value.
    (max(.,0) keeps negatives from turning into fp8 NaNs; numerics validated
    offline at rel_err ~4e-3 vs the 2e-2 gate)

Schedule:
  - fp32 x is never loaded; the residual uses bf16 x and the output is stored
    bf16 (error budget allows it), cutting HBM traffic 5.5MB -> ~2.8MB
  - context accumulation for the left half of the queries (qt 0,1) is
    interleaved into the scores phase pair-by-pair (PSUM: 4 banks scores +
    4 banks ctx-left); the right half runs after from the stored pT8
  - denominator accumulates on two engines (even chunks DVE, odd chunks Pool)
    and merges once at the end
  - ACT/DVE activation table loads and PE warmup happen during the initial DMA
"""

import math
import numpy as np
import ml_dtypes

import concourse.bass as bass
import concourse.tile as tile
from concourse import bacc, mybir
from concourse.bass_utils import run_bass_kernel_spmd

B, C, L = 8, 256, 2048
P = 128                 # partitions
NMC = L // P            # 16 m-chunks (key blocks)
NPAIR = NMC // 2        # 8 DoubleRow pairs
NB = 512                # matmul moving free dim
HALF = 1024
SCALE = float(C) ** -0.5
LN2 = math.log(2.0)
KAPPA = 128.0 * SCALE / LN2     # scores t = kappa * s_raw (baked into mt8 on host)
SHIFT = 2.0                     # global exp shift; cancels in softmax
BD16 = 128.0 / LN2              # bits-per-nat for the bf16 Schraudolph path
C16 = 16248.85                  # centered bf16 bias: bits = BD16*s_eff + C16
WARMUP_MMS = 4

F32 = mybir.dt.float32
BF16 = mybir.dt.bfloat16
F8 = mybir.dt.float8e4
U16 = mybir.dt.uint16
DR = mybir.MatmulPerfMode.DoubleRow

_COMPILED = None


def build_nc():
    nc = bacc.Bacc("TRN2", target_bir_lowering=False, debug=False, num_devices=8)

    x8_d = nc.dram_tensor("x8", [P, 2 * L], F8, kind="ExternalInput").ap()
    xbf_d = nc.dram_tensor("xbf", [C, L], BF16, kind="ExternalInput").ap()
    mt8_d = nc.dram_tensor("mt8", [P, 2 * C], F8, kind="ExternalInput").ap()
    wvu8_d = nc.dram_tensor("wvu8", [P, 2 * 272], F8, kind="ExternalInput").ap()
    bv_d = nc.dram_tensor("bv", [P, 2], F32, kind="ExternalInput").ap()
    out_d = nc.dram_tensor("out", [C, L], BF16, kind="ExternalOutput").ap()

    with tile.TileContext(nc) as tc:
        with (
            tc.tile_pool(name="const", bufs=1) as const,
            tc.tile_pool(name="data", bufs=1) as data,
            tc.tile_pool(name="evict", bufs=4) as evict,
        ):
            # ---- constants / warmup fodder ----
            ones_bf = const.tile([P, NB], BF16)
            nc.vector.memset(ones_bf[:], 1.0)
            ones8 = const.tile([P, 2, 16], F8)
            nc.gpsimd.memset(ones8[:], 1.0)
            tiny = const.tile([P, 2, 16], F32)

            x8 = data.tile([P, 2, L], F8, tag="x8", name="x8")
            xbf = [data.tile([P, L], BF16, tag=f"xbf{c}", name=f"xbf{c}")
                   for c in range(2)]
            mt8 = const.tile([P, 2, C], F8, tag="mt8")
            wvu8 = const.tile([P, 2, 272], F8, tag="wvu8")
            bv_sb = const.tile([P, 2, 1], F32, tag="bv")

            # x8 is host-packed [p, j, l]; 512-col slices interleave on the
            # sync and gpsimd queues (scalar's queue carries the weights and
            # the ACT table load), first slice split for the earliest start
            x8_v = x8_d.rearrange("p (j l) -> p j l", j=2)

            def x8_dma(c0, c1, eng):
                eng.dma_start(out=x8[:, :, c0:c1], in_=x8_v[:, :, c0:c1])

            x8_dma(0, 256, nc.sync)
            x8_dma(256, 512, nc.gpsimd)
            nc.scalar.dma_start(out=mt8[:],
                                in_=mt8_d.rearrange("p (j o) -> p j o", j=2))
            x8_dma(512, 1024, nc.sync)
            x8_dma(1024, 1536, nc.gpsimd)
            nc.scalar.dma_start(out=wvu8[:],
                                in_=wvu8_d.rearrange("p (j o) -> p j o", j=2))
            x8_dma(1536, 2048, nc.sync)
            nc.gpsimd.dma_start(out=bv_sb[:],
                                in_=bv_d.rearrange("p (j o) -> p j o", j=2))

            w8 = data.tile([P, 2, L], F8, tag="w8", name="w8")
            vT_bf = data.tile([P, NMC, C], BF16, tag="vT")
            pT_bf = data.tile([P, NMC, L], BF16, tag="pT")
            b_act = data.tile([P, NMC, 1], F32, tag="b_act")
            dacc = data.tile([P, L], BF16, tag="dacc")
            recip = data.tile([P, L], F32, tag="recip")
            xr = [data.tile([P, L], BF16, tag=f"xr{c}", name=f"xr{c}")
                  for c in range(2)]

            # ---- phase 1: PE warmup + w projection only ----
            with tc.tile_pool(name="psA", bufs=1, space=bass.MemorySpace.PSUM) as psA:
                # warm the activation tables (one-time ~2.7us DMAs) and the PE
                # HAM clock-gate while x streams in
                warm = psA.tile([P, HALF], F32, tag="wp", name="warm", bufs=2)
                nc.scalar.activation(out=tiny[:, 1, :], in_=mt8[:, 0, 0:16],
                                     func=mybir.ActivationFunctionType.Exp,
                                     scale=1.0)
                for i in range(WARMUP_MMS):
                    nc.tensor.matmul(warm[:, 0:NB], ones_bf[:, 0:P],
                                     ones_bf[:], start=True, stop=True)
                nc.tensor.matmul(warm[0:16, 0:16], ones8[:], ones8[:],
                                 start=True, stop=True, perf_mode=DR)

                # w = kappa M x  (kappa baked into mt8 on host); one DoubleRow
                # matmul contracts the full 256 channels. Only the first half
                # of the keys here - the second half rides inside the scores
                # loop so its evicts stay off the startup critical path.
                for oc in range(2):
                    wp = psA.tile([P, HALF], F32, tag="wp", name="wp", bufs=2)
                    for ln in range(2):
                        c0 = ln * NB
                        nc.tensor.matmul(
                            wp[:, ln * NB:(ln + 1) * NB],
                            mt8[:, :, oc * P:(oc + 1) * P],
                            x8[:, :, c0:c0 + NB],
                            start=True, stop=True, perf_mode=DR)
                        nc.vector.tensor_copy(
                            out=w8[:, oc, c0:c0 + NB],
                            in_=wp[:, ln * NB:(ln + 1) * NB])

            # xbf for the residual - only needed by the epilogue; sync queue is
            # idle during the scores phase
            nc.sync.dma_start(out=xbf[0][:], in_=xbf_d[0:P, :])
            nc.gpsimd.dma_start(out=xbf[1][:], in_=xbf_d[P:C, :])

            # ---- phase 2: v-projection + scores + exp + denom + ctx-left ----
            # vp (v-projection) rides inside the scores loop: one DoubleRow
            # matmul per chunk, sharing the PSUM pool with the score tiles so
            # there is no pool-transition barrier before the scores start.
            with tc.tile_pool(name="psCL", bufs=1,
                              space=bass.MemorySpace.PSUM) as psCL:
                ctxL = {(qt, cc): psCL.tile([P, NB], F32, tag=f"cl{qt}{cc}",
                                            name=f"cl{qt}{cc}", bufs=1)
                        for qt in range(2) for cc in range(2)}

                with tc.tile_pool(name="psB", bufs=1,
                                  space=bass.MemorySpace.PSUM) as psB:
                    for mc in range(NMC):
                        if mc == 3:
                            # second half of the w projection (keys 1024:2048),
                            # needed from chunk 8; reuses the s PSUM ring
                            for oc in range(2):
                                for ln in range(2):
                                    c0 = HALF + ln * NB
                                    wh = psB.tile([P, NB], F32, tag="s",
                                                  name="wh", bufs=3)
                                    nc.tensor.matmul(
                                        wh[:], mt8[:, :, oc * P:(oc + 1) * P],
                                        x8[:, :, c0:c0 + NB],
                                        start=True, stop=True, perf_mode=DR)
                                    nc.vector.tensor_copy(
                                        out=w8[:, oc, c0:c0 + NB], in_=wh[:])
                        mrows = slice(mc * P, (mc + 1) * P)
                        # v/ux projection for this key chunk
                        vp = psB.tile([P, 272], F32, tag="vp", name="vp", bufs=1)
                        nc.tensor.matmul(
                            vp[:], x8[:, :, mrows], wvu8[:],
                            start=True, stop=True, perf_mode=DR)
                        nc.vector.tensor_copy(out=vT_bf[:, mc, :],
                                              in_=vp[:, 0:C])
                        nc.vector.tensor_scalar_add(out=b_act[:, mc, :],
                                                    in0=vp[:, C:C + 1],
                                                    scalar1=-SHIFT)
                        # scores + exp, 512 columns at a time
                        for qt in range(4):
                            s = psB.tile([P, NB], F32, tag="s", name="s",
                                         bufs=3)
                            nc.tensor.matmul(
                                s[:], w8[:, :, mrows],
                                x8[:, :, qt * NB:(qt + 1) * NB],
                                start=True, stop=True, perf_mode=DR)
                            nc.scalar.activation(
                                out=pT_bf[:, mc, qt * NB:(qt + 1) * NB],
                                in_=s[:],
                                func=mybir.ActivationFunctionType.Exp,
                                scale=LN2 / 128.0, bias=b_act[:, mc, :])
                        # running denominator (bf16 accumulator on DVE)
                        src = pT_bf[:, mc, :]
                        if mc == 0:
                            nc.vector.tensor_copy(out=dacc[:], in_=src)
                        else:
                            nc.vector.tensor_add(dacc[:], dacc[:], src)
                        # ctx-left accumulates chunk by chunk (bf16)
                        for cc in range(2):
                            for qt in range(2):
                                nc.tensor.matmul(
                                    ctxL[(qt, cc)][:],
                                    vT_bf[:, mc, cc * P:(cc + 1) * P],
                                    pT_bf[:, mc, qt * NB:(qt + 1) * NB],
                                    start=(mc == 0), stop=(mc == NMC - 1))

                # ---- phase 3: denom matmuls + ctx-right + epilogue ----
                with tc.tile_pool(name="psDR", bufs=1,
                                  space=bass.MemorySpace.PSUM) as psDR:
                    def ds_recip(ln):
                        cols = slice(ln * NB, (ln + 1) * NB)
                        ds = psDR.tile([P, NB], F32, tag="ds", name="ds", bufs=2)
                        nc.tensor.matmul(ds[:], ones_bf[:, 0:P], dacc[:, cols],
                                         start=True, stop=True)
                        nc.vector.reciprocal_approx_fast(out=recip[:, cols],
                                                         in_=ds[:])

                    def ctx_mms(ct, qt, cc):
                        for mc in range(NMC):
                            nc.tensor.matmul(
                                ct[:],
                                vT_bf[:, mc, cc * P:(cc + 1) * P],
                                pT_bf[:, mc, qt * NB:(qt + 1) * NB],
                                start=(mc == 0), stop=(mc == NMC - 1))

                    def ct_evict(ct, qt, cc, nsub, qpick):
                        rows = slice(cc * P, (cc + 1) * P)
                        sub = NB // nsub
                        for si in range(nsub):
                            c0 = qt * NB + si * sub
                            cols = slice(c0, c0 + sub)
                            pcols = slice(si * sub, (si + 1) * sub)
                            t = evict.tile([P, sub], F32, tag="t", name="t")
                            nc.vector.tensor_mul(t[:], ct[:, pcols],
                                                 recip[:, cols])
                            o = evict.tile([P, sub], BF16, tag="o", name="o")
                            nc.gpsimd.tensor_add(o[:], t[:], xr[cc][:, cols])
                            deng = nc.sync if (qpick + si) % 2 == 0 else nc.scalar
                            deng.dma_start(out=out_d[rows, cols], in_=o[:])

                    # ds matmuls interleave between ctx-right tiles so the PE
                    # never head-blocks on the reciprocal chain
                    ds_recip(0)
                    ds_recip(1)
                    # residual prep on the idle Pool engine so it cannot
                    # steal DVE slots from the scores pipeline
                    for cc in range(2):
                        nc.gpsimd.tensor_scalar_add(out=xr[cc][:],
                                                    in0=xbf[cc][:],
                                                    scalar1=bv_sb[:, cc, :])
                    ctxR = {}
                    for k, (qt, cc) in enumerate(((2, 0), (2, 1), (3, 0), (3, 1))):
                        ct = psDR.tile([P, NB], F32, tag="cr", name="cr", bufs=2)
                        ctxR[(qt, cc)] = ct
                        ctx_mms(ct, qt, cc)
                        if k == 0:
                            ds_recip(2)
                            ds_recip(3)
                            ct_evict(ctxL[(0, 0)], 0, 0, 1, 0)
                            ct_evict(ctxL[(0, 1)], 0, 1, 1, 1)
                            ct_evict(ctxL[(1, 0)], 1, 0, 1, 0)
                            ct_evict(ctxL[(1, 1)], 1, 1, 1, 1)
                        elif k == 1:
                            ct_evict(ctxR[(2, 0)], 2, 0, 1, 0)
                        elif k == 2:
                            ct_evict(ctxR[(2, 1)], 2, 1, 2, 1)
                    ct_evict(ctxR[(3, 0)], 3, 0, 2, 0)
                    ct_evict(ctxR[(3, 1)], 3, 1, 4, 1)

    nc.compile()
    return nc


def get_compiled():
    global _COMPILED
    if _COMPILED is None:
        _COMPILED = build_nc()
    return _COMPILED


def make_in_maps(inputs):
    f8 = ml_dtypes.float8_e4m3
    x = np.ascontiguousarray(np.asarray(inputs["x"], dtype=np.float32))
    Wq = np.asarray(inputs["Wq"], np.float32)
    Wk = np.asarray(inputs["Wk"], np.float32)
    Wv = np.asarray(inputs["Wv"], np.float32)
    bq = np.asarray(inputs["bq"], np.float32)
    M = Wq.T @ Wk                               # scores_raw = x^T M x
    u = SCALE * (Wk.T @ bq)                     # per-key score bias u.x
    wvu = np.zeros((C, 272), np.float32)
    wvu[:, 0:C] = Wv.T
    wvu[:, C] = u

    def pack(a):
        # [C, F] -> [P, 2*F]: row p holds [a[p, :], a[p+128, :]]
        return np.ascontiguousarray(
            a.reshape(2, P, -1).transpose(1, 0, 2).reshape(P, -1))

    shared = {
        "mt8": pack(KAPPA * M.T).astype(f8),
        "wvu8": pack(wvu).astype(f8),
        "bv": pack(np.asarray(inputs["bv"], np.float32).reshape(C, 1)),
    }
    return [{"x8": pack(x[i]).astype(f8),
             "xbf": x[i].astype(ml_dtypes.bfloat16),
             **shared} for i in range(B)]


def run(inputs, trace=False, **kwargs):
    nc = get_compiled()
    res = run_bass_kernel_spmd(nc, make_in_maps(inputs),
                               core_ids=list(range(B)), trace=trace, **kwargs)
    out = np.stack([res.results[i]["out"] for i in range(B)], axis=0)
    return out.astype(np.float32), res


def kernel(**inputs):
    out, _ = run(inputs)
    return out


# revision 22
# speedup vs baseline: 1.5400x; 1.5400x over previous
"""Trainium2 Bass kernel for AttentionBlock (B=8, C=256, L=2048), data-parallel
over batch across 8 NeuronCores.

Math (one batch per core, x: [C, L]):
    t^T   = w8^T x8            w8 = fp8(kappa M x),  M = Wq^T Wk,  kappa = 8*SCALE/ln2
    pT    = exp-ish(t)         [m, l], m on partitions; global shift cancels in softmax
    denom = ones^T acc(pT)     (two running bf16 accumulators, DVE + Pool)
    ctx   = vT8^T pT           vT8 = fp8(x^T Wv^T); ux (per-key bq.Wk x) rides along as
                               a 257th output column of the same projection
    out   = ctx * (1/denom) + (bf16(x) + bv)

All heavy matmuls run in fp8e4 with perf_mode=DoubleRow: operands are packed
[128, 2, free] so one instruction contracts 256 deep (2 k-tiles), ~1.5x bf16
throughput at FD>=512.

exp is split across two engines:
  - ACT chunks: nc.scalar.activation(Exp, scale=ln2/8, bias=ux-shift) -> fp8 direct
  - DVE chunks: Schraudolph-in-fp8: bits = clamp(t + b_dve, 0) as uint8, where
    b_dve = (8/ln2)(ux-shift) + 56; the uint8 bit pattern IS the fp8 exp value.
    (max(.,0) keeps negatives from turning into fp8 NaNs; numerics validated
    offline at rel_err ~4e-3 vs the 2e-2 gate)

Schedule:
  - fp32 x is never loaded; the residual uses bf16 x and the output is stored
    bf16 (error budget allows it), cutting HBM traffic 5.5MB -> ~2.8MB
  - context accumulation for the left half of the queries (qt 0,1) is
    interleaved into the scores phase pair-by-pair (PSUM: 4 banks scores +
    4 banks ctx-left); the right half runs after from the stored pT8
  - denominator accumulates on two engines (even chunks DVE, odd chunks Pool)
    and merges once at the end
  - ACT/DVE activation table loads and PE warmup happen during the initial DMA
"""

import math
import numpy as np
import ml_dtypes

import concourse.bass as bass
import concourse.tile as tile
from concourse import bacc, mybir
from concourse.bass_utils import run_bass_kernel_spmd

B, C, L = 8, 256, 2048
P = 128                 # partitions
NMC = L // P            # 16 m-chunks (key blocks)
NPAIR = NMC // 2        # 8 DoubleRow pairs
NB = 512                # matmul moving free dim
HALF = 1024
SCALE = float(C) ** -0.5
LN2 = math.log(2.0)
KAPPA = 128.0 * SCALE / LN2     # scores t = kappa * s_raw (baked into mt8 on host)
SHIFT = 2.0                     # global exp shift; cancels in softmax
BD16 = 128.0 / LN2              # bits-per-nat for the bf16 Schraudolph path
C16 = 16248.85                  # centered bf16 bias: bits = BD16*s_eff + C16
WARMUP_MMS = 4

F32 = mybir.dt.float32
BF16 = mybir.dt.bfloat16
F8 = mybir.dt.float8e4
U16 = mybir.dt.uint16
DR = mybir.MatmulPerfMode.DoubleRow

_COMPILED = None


def build_nc():
    nc = bacc.Bacc("TRN2", target_bir_lowering=False, debug=False, num_devices=8)

    x8_d = nc.dram_tensor("x8", [P, 2 * L], F8, kind="ExternalInput").ap()
    xbf_d = nc.dram_tensor("xbf", [C, L], BF16, kind="ExternalInput").ap()
    mt8_d = nc.dram_tensor("mt8", [P, 2 * C], F8, kind="ExternalInput").ap()
    wvu8_d = nc.dram_tensor("wvu8", [P, 2 * 272], F8, kind="ExternalInput").ap()
    bv_d = nc.dram_tensor("bv", [P, 2], F32, kind="ExternalInput").ap()
    out_d = nc.dram_tensor("out", [C, L], BF16, kind="ExternalOutput").ap()

    with tile.TileContext(nc) as tc:
        with (
            tc.tile_pool(name="const", bufs=1) as const,
            tc.tile_pool(name="data", bufs=1) as data,
            tc.tile_pool(name="evict", bufs=4) as evict,
        ):
            # ---- constants / warmup fodder ----
            ones_bf = const.tile([P, NB], BF16)
            nc.vector.memset(ones_bf[:], 1.0)
            ones8 = const.tile([P, 2, 16], F8)
            nc.gpsimd.memset(ones8[:], 1.0)
            tiny = const.tile([P, 2, 16], F32)

            x8 = data.tile([P, 2, L], F8, tag="x8", name="x8")
            xbf = [data.tile([P, L], BF16, tag=f"xbf{c}", name=f"xbf{c}")
                   for c in range(2)]
            mt8 = const.tile([P, 2, C], F8, tag="mt8")
            wvu8 = const.tile([P, 2, 272], F8, tag="wvu8")
            bv_sb = const.tile([P, 2, 1], F32, tag="bv")

            # x8 is host-packed [p, j, l]; 512-col slices interleave on the
            # sync and gpsimd queues (scalar's queue carries the weights and
            # the ACT table load), first slice split for the earliest start
            x8_v = x8_d.rearrange("p (j l) -> p j l", j=2)

            def x8_dma(c0, c1, eng):
                eng.dma_start(out=x8[:, :, c0:c1], in_=x8_v[:, :, c0:c1])

            x8_dma(0, 256, nc.sync)
            x8_dma(256, 512, nc.gpsimd)
            nc.scalar.dma_start(out=mt8[:],
                                in_=mt8_d.rearrange("p (j o) -> p j o", j=2))
            x8_dma(512, 1024, nc.sync)
            x8_dma(1024, 1536, nc.gpsimd)
            nc.scalar.dma_start(out=wvu8[:],
                                in_=wvu8_d.rearrange("p (j o) -> p j o", j=2))
            x8_dma(1536, 2048, nc.sync)
            nc.gpsimd.dma_start(out=bv_sb[:],
                                in_=bv_d.rearrange("p (j o) -> p j o", j=2))

            w8 = data.tile([P, 2, L], F8, tag="w8", name="w8")
            vT_bf = data.tile([P, NMC, C], BF16, tag="vT")
            pT_bf = data.tile([P, NMC, L], BF16, tag="pT")
            b_act = data.tile([P, NMC, 1], F32, tag="b_act")
            b16 = data.tile([P, NMC, 1], F32, tag="b16")
            bv_late = data.tile([P, 2, 1], F32, tag="bv_late")
            dacc = data.tile([P, L], BF16, tag="dacc")
            recip = data.tile([P, L], F32, tag="recip")
            xr = [data.tile([P, L], BF16, tag=f"xr{c}", name=f"xr{c}")
                  for c in range(2)]

            # ---- phase 1: PE warmup + w projection only ----
            with tc.tile_pool(name="psA", bufs=1, space=bass.MemorySpace.PSUM) as psA:
                # warm the activation tables (one-time ~2.7us DMAs) and the PE
                # HAM clock-gate while x streams in
                warm = psA.tile([P, HALF], F32, tag="wp", name="warm", bufs=2)
                nc.scalar.activation(out=tiny[:, 1, :], in_=mt8[:, 0, 0:16],
                                     func=mybir.ActivationFunctionType.Exp,
                                     scale=1.0)
                for i in range(WARMUP_MMS):
                    nc.tensor.matmul(warm[:, 0:NB], ones_bf[:, 0:P],
                                     ones_bf[:], start=True, stop=True)
                nc.tensor.matmul(warm[0:16, 0:16], ones8[:], ones8[:],
                                 start=True, stop=True, perf_mode=DR)

                # w = kappa M x  (kappa baked into mt8 on host); one DoubleRow
                # matmul contracts the full 256 channels
                for h in range(2):
                    for oc in range(2):
                        wp = psA.tile([P, HALF], F32, tag="wp", name="wp", bufs=2)
                        for ln in range(2):
                            c0 = h * HALF + ln * NB
                            nc.tensor.matmul(
                                wp[:, ln * NB:(ln + 1) * NB],
                                mt8[:, :, oc * P:(oc + 1) * P],
                                x8[:, :, c0:c0 + NB],
                                start=True, stop=True, perf_mode=DR)
                            nc.vector.tensor_copy(
                                out=w8[:, oc, c0:c0 + NB],
                                in_=wp[:, ln * NB:(ln + 1) * NB])

            # xbf for the residual - only needed by the epilogue; sync queue is
            # idle during the scores phase
            nc.sync.dma_start(out=xbf[0][:], in_=xbf_d[0:P, :])
            nc.gpsimd.dma_start(out=xbf[1][:], in_=xbf_d[P:C, :])

            # ---- phase 2: v-projection + scores + exp + denom + ctx-left ----
            # vp (v-projection) rides inside the scores loop: one DoubleRow
            # matmul per chunk, sharing the PSUM pool with the score tiles so
            # there is no pool-transition barrier before the scores start.
            with tc.tile_pool(name="psCL", bufs=1,
                              space=bass.MemorySpace.PSUM) as psCL:
                ctxL = {(qt, cc): psCL.tile([P, NB], F32, tag=f"cl{qt}{cc}",
                                            name=f"cl{qt}{cc}", bufs=1)
                        for qt in range(2) for cc in range(2)}

                with tc.tile_pool(name="psB", bufs=1,
                                  space=bass.MemorySpace.PSUM) as psB:
                    for mc in range(NMC):
                        mrows = slice(mc * P, (mc + 1) * P)
                        # v/ux projection for this key chunk
                        vp = psB.tile([P, 272], F32, tag="vp", name="vp", bufs=1)
                        nc.tensor.matmul(
                            vp[:], x8[:, :, mrows], wvu8[:],
                            start=True, stop=True, perf_mode=DR)
                        nc.vector.tensor_copy(out=vT_bf[:, mc, :],
                                              in_=vp[:, 0:C])
                        nc.scalar.activation(
                            out=b_act[:, mc, :], in_=vp[:, C:C + 1],
                            func=mybir.ActivationFunctionType.Copy,
                            bias=-SHIFT)
                        nc.scalar.activation(
                            out=b16[:, mc, :], in_=vp[:, C:C + 1],
                            func=mybir.ActivationFunctionType.Copy,
                            scale=BD16, bias=C16 - BD16 * SHIFT)
                        # scores + exp, 512 columns at a time
                        for qt in range(4):
                            s = psB.tile([P, NB], F32, tag="s", name="s",
                                         bufs=3)
                            nc.tensor.matmul(
                                s[:], w8[:, :, mrows],
                                x8[:, :, qt * NB:(qt + 1) * NB],
                                start=True, stop=True, perf_mode=DR)
                            if qt < 3:
                                nc.scalar.activation(
                                    out=pT_bf[:, mc, qt * NB:(qt + 1) * NB],
                                    in_=s[:],
                                    func=mybir.ActivationFunctionType.Exp,
                                    scale=LN2 / 128.0, bias=b_act[:, mc, :])
                            else:
                                nc.vector.tensor_scalar(
                                    out=pT_bf[:, mc,
                                              qt * NB:(qt + 1) * NB].bitcast(U16),
                                    in0=s[:], scalar1=b16[:, mc, :],
                                    scalar2=0.0,
                                    op0=mybir.AluOpType.add,
                                    op1=mybir.AluOpType.max)
                        # running denominator (bf16 accumulator on DVE)
                        src = pT_bf[:, mc, :]
                        if mc == 0:
                            nc.vector.tensor_copy(out=dacc[:], in_=src)
                        else:
                            nc.vector.tensor_add(dacc[:], dacc[:], src)
                        # ctx-left accumulates chunk by chunk (bf16)
                        for cc in range(2):
                            for qt in range(2):
                                nc.tensor.matmul(
                                    ctxL[(qt, cc)][:],
                                    vT_bf[:, mc, cc * P:(cc + 1) * P],
                                    pT_bf[:, mc, qt * NB:(qt + 1) * NB],
                                    start=(mc == 0), stop=(mc == NMC - 1))

                # ---- phase 3: denom matmuls + ctx-right + epilogue ----
                with tc.tile_pool(name="psDR", bufs=1,
                                  space=bass.MemorySpace.PSUM) as psDR:
                    def ds_recip(ln):
                        cols = slice(ln * NB, (ln + 1) * NB)
                        ds = psDR.tile([P, NB], F32, tag="ds", name="ds", bufs=2)
                        nc.tensor.matmul(ds[:], ones_bf[:, 0:P], dacc[:, cols],
                                         start=True, stop=True)
                        nc.vector.reciprocal_approx_fast(out=recip[:, cols],
                                                         in_=ds[:])
                        return ds

                    def ctx_mms(ct, qt, cc):
                        for mc in range(NMC):
                            nc.tensor.matmul(
                                ct[:],
                                vT_bf[:, mc, cc * P:(cc + 1) * P],
                                pT_bf[:, mc, qt * NB:(qt + 1) * NB],
                                start=(mc == 0), stop=(mc == NMC - 1))

                    def ct_evict(ct, qt, cc, nsub, qpick):
                        rows = slice(cc * P, (cc + 1) * P)
                        sub = NB // nsub
                        for si in range(nsub):
                            c0 = qt * NB + si * sub
                            cols = slice(c0, c0 + sub)
                            pcols = slice(si * sub, (si + 1) * sub)
                            t = evict.tile([P, sub], F32, tag="t", name="t")
                            nc.vector.tensor_mul(t[:], ct[:, pcols],
                                                 recip[:, cols])
                            o = evict.tile([P, sub], BF16, tag="o", name="o")
                            nc.gpsimd.tensor_add(o[:], t[:], xr[cc][:, cols])
                            deng = nc.sync if (qpick + si) % 2 == 0 else nc.scalar
                            deng.dma_start(out=out_d[rows, cols], in_=o[:])

                    # ds matmuls interleave between ctx-right tiles so the PE
                    # never head-blocks on the reciprocal chain
                    ds0 = ds_recip(0)
                    ds_recip(1)
                    # residual prep on the phase-3-idle ACT engine; the
                    # bv_late indirection pins it behind the denominator so
                    # the scheduler cannot hoist it into the scores phase
                    nc.vector.tensor_scalar(out=bv_late[:], in0=bv_sb[:],
                                            scalar1=ds0[:, 0:1],
                                            scalar2=ds0[:, 0:1],
                                            op0=mybir.AluOpType.add,
                                            op1=mybir.AluOpType.subtract)
                    for cc in range(2):
                        nc.scalar.add(out=xr[cc][:], in_=xbf[cc][:],
                                      add=bv_late[:, cc, :])
                    ctxR = {}
                    for k, (qt, cc) in enumerate(((2, 0), (2, 1), (3, 0), (3, 1))):
                        ct = psDR.tile([P, NB], F32, tag="cr", name="cr", bufs=2)
                        ctxR[(qt, cc)] = ct
                        ctx_mms(ct, qt, cc)
                        if k == 0:
                            ds_recip(2)
                            ds_recip(3)
                            ct_evict(ctxL[(0, 0)], 0, 0, 1, 0)
                            ct_evict(ctxL[(0, 1)], 0, 1, 1, 1)
                            ct_evict(ctxL[(1, 0)], 1, 0, 1, 0)
                            ct_evict(ctxL[(1, 1)], 1, 1, 1, 1)
                        elif k == 1:
                            ct_evict(ctxR[(2, 0)], 2, 0, 1, 0)
                        elif k == 2:
                            ct_evict(ctxR[(2, 1)], 2, 1, 2, 1)
                    ct_evict(ctxR[(3, 0)], 3, 0, 2, 0)
                    ct_evict(ctxR[(3, 1)], 3, 1, 4, 1)

    nc.compile()
    return nc


def get_compiled():
    global _COMPILED
    if _COMPILED is None:
        _COMPILED = build_nc()
    return _COMPILED


def make_in_maps(inputs):
    f8 = ml_dtypes.float8_e4m3
    x = np.ascontiguousarray(np.asarray(inputs["x"], dtype=np.float32))
    Wq = np.asarray(inputs["Wq"], np.float32)
    Wk = np.asarray(inputs["Wk"], np.float32)
    Wv = np.asarray(inputs["Wv"], np.float32)
    bq = np.asarray(inputs["bq"], np.float32)
    M = Wq.T @ Wk                               # scores_raw = x^T M x
    u = SCALE * (Wk.T @ bq)                     # per-key score bias u.x
    wvu = np.zeros((C, 272), np.float32)
    wvu[:, 0:C] = Wv.T
    wvu[:, C] = u

    def pack(a):
        # [C, F] -> [P, 2*F]: row p holds [a[p, :], a[p+128, :]]
        return np.ascontiguousarray(
            a.reshape(2, P, -1).transpose(1, 0, 2).reshape(P, -1))

    shared = {
        "mt8": pack(KAPPA * M.T).astype(f8),
        "wvu8": pack(wvu).astype(f8),
        "bv": pack(np.asarray(inputs["bv"], np.float32).reshape(C, 1)),
    }
    return [{"x8": pack(x[i]).astype(f8),
             "xbf": x[i].astype(ml_dtypes.bfloat16),
             **shared} for i in range(B)]


def run(inputs, trace=False, **kwargs):
    nc = get_compiled()
    res = run_bass_kernel_spmd(nc, make_in_maps(inputs),
                               core_ids=list(range(B)), trace=trace, **kwargs)
    out = np.stack([res.results[i]["out"] for i in range(B)], axis=0)
    return out.astype(np.float32), res


def kernel(**inputs):
    out, _ = run(inputs)
    return out


# revision 23
# speedup vs baseline: 1.6059x; 1.0428x over previous
"""Trainium2 Bass kernel for AttentionBlock (B=8, C=256, L=2048), data-parallel
over batch across 8 NeuronCores.

Math (one batch per core, x: [C, L]):
    t^T   = w8^T x8            w8 = fp8(kappa M x),  M = Wq^T Wk,  kappa = 128*SCALE/ln2
    pT    = exp(t*ln2/128 + ux - shift)   [m, l], m on partitions; the global
                               shift cancels in softmax
    denom = ones^T acc(pT)     (running bf16 accumulator on DVE)
    ctx   = vT^T pT            vT = x^T Wv^T in bf16; ux (per-key bq.Wk x)
                               rides along as a 257th output column of the
                               same projection
    out   = ctx * (1/denom) + (bf16(x) + bv)

The C=256 contractions (w projection, v projection, scores) run in fp8e4 with
perf_mode=DoubleRow: operands packed [128, 2, free], one instruction contracts
256 deep. On this silicon DoubleRow matches bf16 ALU throughput, so its win is
instruction/LDWEIGHTS count, and pT/vT stay bf16 (fp8 elementwise ops on DVE
run at 1x and dominate otherwise; measured).

Schedule:
  - fp32 x is never loaded; the residual uses bf16 x and the output is stored
    bf16 (error budget allows it: rel_err ~3.8e-3 vs the 2e-2 gate)
  - the v projection rides inside the scores loop (one DoubleRow matmul per
    key chunk) sharing the PSUM pool with the score tiles, so there is no
    pool-transition barrier before the scores start
  - context accumulation for the left half of the queries (qt 0,1) is
    interleaved into the scores phase chunk by chunk (PSUM: 4 banks scores/vp
    + 4 banks ctx-left); the right half runs after from the stored pT
  - exp runs on ACT (4 x 512-wide slices per chunk, ~2.7us) pacing the PE
    (~2.7us/chunk); the denominator accumulates on DVE in bf16 (2x mode)
  - the residual prep is pinned behind the denominator matmul via a dummy
    data dependency so the scheduler cannot hoist it into the scores-phase
    DVE stream (DVE executes strictly in order; one early op delays every
    later consumer)
  - ACT/DVE table loads and PE warmup happen during the initial DMA
"""

import math
import numpy as np
import ml_dtypes

import concourse.bass as bass
import concourse.tile as tile
from concourse import bacc, mybir
from concourse.bass_utils import run_bass_kernel_spmd

B, C, L = 8, 256, 2048
P = 128                 # partitions
NMC = L // P            # 16 m-chunks (key blocks)
NPAIR = NMC // 2
NB = 512                # matmul moving free dim
HALF = 1024
SCALE = float(C) ** -0.5
LN2 = math.log(2.0)
KAPPA = 128.0 * SCALE / LN2     # scores t = kappa * s_raw (baked into mt8 on host)
SHIFT = 2.0                     # global exp shift; cancels in softmax
WARMUP_MMS = 4

F32 = mybir.dt.float32
BF16 = mybir.dt.bfloat16
F8 = mybir.dt.float8e4
DR = mybir.MatmulPerfMode.DoubleRow

_COMPILED = None


def build_nc():
    nc = bacc.Bacc("TRN2", target_bir_lowering=False, debug=False, num_devices=8)

    x8_d = nc.dram_tensor("x8", [C, L], F8, kind="ExternalInput").ap()
    xbf_d = nc.dram_tensor("xbf", [C, L], BF16, kind="ExternalInput").ap()
    mt8_d = nc.dram_tensor("mt8", [C, C], F8, kind="ExternalInput").ap()
    wvu8_d = nc.dram_tensor("wvu8", [C, 272], F8, kind="ExternalInput").ap()
    bv_d = nc.dram_tensor("bv", [C, 1], F32, kind="ExternalInput").ap()
    out_d = nc.dram_tensor("out", [C, L], BF16, kind="ExternalOutput").ap()

    with tile.TileContext(nc) as tc:
        with (
            tc.tile_pool(name="const", bufs=1) as const,
            tc.tile_pool(name="data", bufs=1) as data,
            tc.tile_pool(name="evict", bufs=4) as evict,
        ):
            # ---- constants / warmup fodder ----
            ones_bf = const.tile([P, NB], BF16)
            nc.vector.memset(ones_bf[:], 1.0)
            ones8 = const.tile([P, 2, 16], F8)
            nc.gpsimd.memset(ones8[:], 1.0)
            tiny = const.tile([P, 2, 16], F32)

            x8 = data.tile([P, 2, L], F8, tag="x8", name="x8")
            xbf = [data.tile([P, L], BF16, tag=f"xbf{c}", name=f"xbf{c}")
                   for c in range(2)]
            mt8 = const.tile([P, 2, C], F8, tag="mt8")
            wvu8 = const.tile([P, 2, 272], F8, tag="wvu8")
            bv_sb = const.tile([P, 2, 1], F32, tag="bv")

            # first l-slice of x8 on several queues, then weights, then rest
            def x8_dma(c0, c1, eng):
                cols = slice(c0, c1)
                eng.dma_start(out=x8[:, :, cols],
                              in_=x8_d[:, cols].rearrange("(j p) l -> p j l",
                                                          p=P))

            x8_dma(0, 768, nc.sync)
            x8_dma(768, 1536, nc.scalar)
            x8_dma(1536, 2048, nc.gpsimd)
            nc.sync.dma_start(out=mt8[:],
                              in_=mt8_d.rearrange("(j p) o -> p j o", p=P))
            nc.scalar.dma_start(out=wvu8[:],
                                in_=wvu8_d.rearrange("(j p) o -> p j o", p=P))
            nc.scalar.dma_start(out=bv_sb[:],
                                in_=bv_d.rearrange("(j p) o -> p j o", p=P))

            w8 = data.tile([P, 2, L], F8, tag="w8", name="w8")
            vT_bf = data.tile([P, NMC, C], BF16, tag="vT")
            pT_bf = data.tile([P, NMC, L], BF16, tag="pT")
            b_act = data.tile([P, NMC, 1], F32, tag="b_act")
            bv_late = data.tile([P, 2, 1], F32, tag="bv_late")
            dacc = data.tile([P, L], BF16, tag="dacc")
            recip = data.tile([P, L], F32, tag="recip")
            xr = [data.tile([P, L], BF16, tag=f"xr{c}", name=f"xr{c}")
                  for c in range(2)]

            # ---- phase 1: PE warmup + w projection ----
            with tc.tile_pool(name="psA", bufs=1, space=bass.MemorySpace.PSUM) as psA:
                # warm the activation tables (one-time ~2.7us DMAs) and the PE
                # HAM clock-gate while x streams in
                warm = psA.tile([P, HALF], F32, tag="wp", name="warm", bufs=2)
                nc.vector.memset(tiny[:, 0, :], 1.0)
                nc.scalar.activation(out=tiny[:, 1, :], in_=tiny[:, 0, :],
                                     func=mybir.ActivationFunctionType.Exp,
                                     scale=1.0)
                nc.vector.reciprocal_approx_fast(out=tiny[:, 1, :],
                                                 in_=tiny[:, 0, :])
                for i in range(WARMUP_MMS):
                    nc.tensor.matmul(warm[:, 0:NB], ones_bf[:, 0:P],
                                     ones_bf[:], start=True, stop=True)
                nc.tensor.matmul(warm[0:16, 0:16], ones8[:], ones8[:],
                                 start=True, stop=True, perf_mode=DR)

                # w = kappa M x  (kappa baked into mt8 on host); one DoubleRow
                # matmul contracts the full 256 channels
                for h in range(2):
                    hcols = slice(h * HALF, (h + 1) * HALF)
                    for oc in range(2):
                        wp = psA.tile([P, HALF], F32, tag="wp", name="wp",
                                      bufs=2)
                        for ln in range(2):
                            c0 = h * HALF + ln * NB
                            nc.tensor.matmul(
                                wp[:, ln * NB:(ln + 1) * NB],
                                mt8[:, :, oc * P:(oc + 1) * P],
                                x8[:, :, c0:c0 + NB],
                                start=True, stop=True, perf_mode=DR)
                        nc.vector.tensor_copy(out=w8[:, oc, hcols], in_=wp[:])

            # xbf for the residual - only needed by the epilogue; these queues
            # are idle during the scores phase
            nc.sync.dma_start(out=xbf[0][:], in_=xbf_d[0:P, :])
            nc.gpsimd.dma_start(out=xbf[1][:], in_=xbf_d[P:C, :])

            # ---- phase 2: v-projection + scores + exp + denom + ctx-left ----
            with tc.tile_pool(name="psCL", bufs=1,
                              space=bass.MemorySpace.PSUM) as psCL:
                ctxL = {(qt, cc): psCL.tile([P, NB], F32, tag=f"cl{qt}{cc}",
                                            name=f"cl{qt}{cc}", bufs=1)
                        for qt in range(2) for cc in range(2)}

                with tc.tile_pool(name="psB", bufs=1,
                                  space=bass.MemorySpace.PSUM) as psB:
                    for mc in range(NMC):
                        mrows = slice(mc * P, (mc + 1) * P)
                        # v/ux projection for this key chunk
                        vp = psB.tile([P, 272], F32, tag="vp", name="vp", bufs=1)
                        nc.tensor.matmul(
                            vp[:], x8[:, :, mrows], wvu8[:],
                            start=True, stop=True, perf_mode=DR)
                        nc.vector.tensor_copy(out=vT_bf[:, mc, :],
                                              in_=vp[:, 0:C])
                        nc.vector.tensor_scalar_add(out=b_act[:, mc, :],
                                                    in0=vp[:, C:C + 1],
                                                    scalar1=-SHIFT)
                        # scores + exp, 512 columns at a time
                        for qt in range(4):
                            s = psB.tile([P, NB], F32, tag="s", name="s",
                                         bufs=3)
                            nc.tensor.matmul(
                                s[:], w8[:, :, mrows],
                                x8[:, :, qt * NB:(qt + 1) * NB],
                                start=True, stop=True, perf_mode=DR)
                            nc.scalar.activation(
                                out=pT_bf[:, mc, qt * NB:(qt + 1) * NB],
                                in_=s[:],
                                func=mybir.ActivationFunctionType.Exp,
                                scale=LN2 / 128.0, bias=b_act[:, mc, :])
                        # running denominator (bf16 accumulator on DVE)
                        src = pT_bf[:, mc, :]
                        if mc == 0:
                            nc.vector.tensor_copy(out=dacc[:], in_=src)
                        else:
                            nc.vector.tensor_add(dacc[:], dacc[:], src)
                        # ctx-left accumulates chunk by chunk (bf16)
                        for cc in range(2):
                            for qt in range(2):
                                nc.tensor.matmul(
                                    ctxL[(qt, cc)][:],
                                    vT_bf[:, mc, cc * P:(cc + 1) * P],
                                    pT_bf[:, mc, qt * NB:(qt + 1) * NB],
                                    start=(mc == 0), stop=(mc == NMC - 1))

                # ---- phase 3: denom matmuls + ctx-right + epilogue ----
                with tc.tile_pool(name="psDR", bufs=1,
                                  space=bass.MemorySpace.PSUM) as psDR:
                    def ds_recip(ln):
                        cols = slice(ln * NB, (ln + 1) * NB)
                        ds = psDR.tile([P, NB], F32, tag="ds", name="ds", bufs=2)
                        nc.tensor.matmul(ds[:], ones_bf[:, 0:P], dacc[:, cols],
                                         start=True, stop=True)
                        nc.vector.reciprocal_approx_fast(out=recip[:, cols],
                                                         in_=ds[:])
                        return ds

                    def ctx_mms(ct, qt, cc):
                        for mc in range(NMC):
                            nc.tensor.matmul(
                                ct[:],
                                vT_bf[:, mc, cc * P:(cc + 1) * P],
                                pT_bf[:, mc, qt * NB:(qt + 1) * NB],
                                start=(mc == 0), stop=(mc == NMC - 1))

                    def ct_evict(ct, qt, cc, nsub, qpick):
                        rows = slice(cc * P, (cc + 1) * P)
                        sub = NB // nsub
                        for si in range(nsub):
                            c0 = qt * NB + si * sub
                            cols = slice(c0, c0 + sub)
                            pcols = slice(si * sub, (si + 1) * sub)
                            t = evict.tile([P, sub], F32, tag="t", name="t")
                            nc.vector.tensor_mul(t[:], ct[:, pcols],
                                                 recip[:, cols])
                            o = evict.tile([P, sub], BF16, tag="o", name="o")
                            nc.gpsimd.tensor_add(o[:], t[:], xr[cc][:, cols])
                            deng = nc.sync if (qpick + si) % 2 == 0 else nc.scalar
                            deng.dma_start(out=out_d[rows, cols], in_=o[:])

                    # ds matmuls interleave between ctx-right tiles so the PE
                    # never head-blocks on the reciprocal chain
                    ds0 = ds_recip(0)
                    ds_recip(1)
                    # residual prep, pinned behind the denominator so the
                    # scheduler cannot hoist it into the scores-phase DVE queue
                    nc.vector.tensor_scalar(out=bv_late[:], in0=bv_sb[:],
                                            scalar1=ds0[:, 0:1],
                                            scalar2=ds0[:, 0:1],
                                            op0=mybir.AluOpType.add,
                                            op1=mybir.AluOpType.subtract)
                    for cc in range(2):
                        nc.scalar.add(out=xr[cc][:], in_=xbf[cc][:],
                                      add=bv_late[:, cc, :])
                    ctxR = {}
                    for k, (qt, cc) in enumerate(((2, 0), (2, 1), (3, 0), (3, 1))):
                        ct = psDR.tile([P, NB], F32, tag="cr", name="cr", bufs=2)
                        ctxR[(qt, cc)] = ct
                        ctx_mms(ct, qt, cc)
                        if k == 0:
                            ds_recip(2)
                            ds_recip(3)
                            ct_evict(ctxL[(0, 0)], 0, 0, 1, 0)
                            ct_evict(ctxL[(0, 1)], 0, 1, 1, 1)
                        elif k == 1:
                            ct_evict(ctxL[(1, 0)], 1, 0, 1, 0)
                            ct_evict(ctxL[(1, 1)], 1, 1, 1, 1)
                            ct_evict(ctxR[(2, 0)], 2, 0, 1, 0)
                        elif k == 2:
                            ct_evict(ctxR[(2, 1)], 2, 1, 2, 1)
                    ct_evict(ctxR[(3, 0)], 3, 0, 2, 0)
                    ct_evict(ctxR[(3, 1)], 3, 1, 4, 1)

    nc.compile()
    return nc


def get_compiled():
    global _COMPILED
    if _COMPILED is None:
        _COMPILED = build_nc()
    return _COMPILED


def make_in_maps(inputs):
    f8 = ml_dtypes.float8_e4m3
    x = np.ascontiguousarray(np.asarray(inputs["x"], dtype=np.float32))
    Wq = np.asarray(inputs["Wq"], np.float32)
    Wk = np.asarray(inputs["Wk"], np.float32)
    Wv = np.asarray(inputs["Wv"], np.float32)
    bq = np.asarray(inputs["bq"], np.float32)
    M = Wq.T @ Wk                               # scores_raw = x^T M x
    u = SCALE * (Wk.T @ bq)                     # per-key score bias u.x
    wvu = np.zeros((C, 272), np.float32)
    wvu[:, 0:C] = Wv.T
    wvu[:, C] = u
    shared = {
        "mt8": np.ascontiguousarray(KAPPA * M.T).astype(f8),
        "wvu8": wvu.astype(f8),
        "bv": np.asarray(inputs["bv"], np.float32).reshape(C, 1),
    }
    return [{"x8": x[i].astype(f8), "xbf": x[i].astype(ml_dtypes.bfloat16),
             **shared} for i in range(B)]


def run(inputs, trace=False, **kwargs):
    nc = get_compiled()
    res = run_bass_kernel_spmd(nc, make_in_maps(inputs),
                               core_ids=list(range(B)), trace=trace, **kwargs)
    out = np.stack([res.results[i]["out"] for i in range(B)], axis=0)
    return out.astype(np.float32), res


def kernel(**inputs):
    out, _ = run(inputs)
    return out


# revision 24
# speedup vs baseline: 1.6181x; 1.0076x over previous
"""Trainium2 Bass kernel for AttentionBlock (B=8, C=256, L=2048), data-parallel
over batch across 8 NeuronCores.

Math (one batch per core, x: [C, L]):
    t^T   = w8^T x8            w8 = fp8(kappa M x),  M = Wq^T Wk,  kappa = 128*SCALE/ln2
    pT    = exp(t*ln2/128 + ux - shift)   [m, l], m on partitions; the global
                               shift cancels in softmax
    denom = ones^T acc(pT)     (running bf16 accumulator on DVE)
    ctx   = vT^T pT            vT = x^T Wv^T in bf16; ux (per-key bq.Wk x)
                               rides along as a 257th output column of the
                               same projection
    out   = ctx * (1/denom) + (bf16(x) + bv)

The C=256 contractions (w projection, v projection, scores) run in fp8e4 with
perf_mode=DoubleRow: operands packed [128, 2, free], one instruction contracts
256 deep. On this silicon DoubleRow matches bf16 ALU throughput, so its win is
instruction/LDWEIGHTS count, and pT/vT stay bf16 (fp8 elementwise ops on DVE
run at 1x and dominate otherwise; measured).

Schedule:
  - fp32 x is never loaded; the residual uses bf16 x and the output is stored
    bf16 (error budget allows it: rel_err ~3.8e-3 vs the 2e-2 gate)
  - the v projection rides inside the scores loop (one DoubleRow matmul per
    key chunk) sharing the PSUM pool with the score tiles, so there is no
    pool-transition barrier before the scores start
  - context accumulation for the left half of the queries (qt 0,1) is
    interleaved into the scores phase chunk by chunk (PSUM: 4 banks scores/vp
    + 4 banks ctx-left); the right half runs after from the stored pT
  - exp runs on ACT (4 x 512-wide slices per chunk, ~2.7us) pacing the PE
    (~2.7us/chunk); the denominator accumulates on DVE in bf16 (2x mode)
  - the residual prep is pinned behind the denominator matmul via a dummy
    data dependency so the scheduler cannot hoist it into the scores-phase
    DVE stream (DVE executes strictly in order; one early op delays every
    later consumer)
  - ACT/DVE table loads and PE warmup happen during the initial DMA
"""

import math
import numpy as np
import ml_dtypes

import concourse.bass as bass
import concourse.tile as tile
from concourse import bacc, mybir
from concourse.bass_utils import run_bass_kernel_spmd

B, C, L = 8, 256, 2048
P = 128                 # partitions
NMC = L // P            # 16 m-chunks (key blocks)
NPAIR = NMC // 2
NB = 512                # matmul moving free dim
HALF = 1024
SCALE = float(C) ** -0.5
LN2 = math.log(2.0)
KAPPA = 128.0 * SCALE / LN2     # scores t = kappa * s_raw (baked into mt8 on host)
SHIFT = 2.0                     # global exp shift; cancels in softmax
WARMUP_MMS = 4

F32 = mybir.dt.float32
BF16 = mybir.dt.bfloat16
F8 = mybir.dt.float8e4
DR = mybir.MatmulPerfMode.DoubleRow

_COMPILED = None


def build_nc():
    nc = bacc.Bacc("TRN2", target_bir_lowering=False, debug=False, num_devices=8)

    x8_d = nc.dram_tensor("x8", [C, L], F8, kind="ExternalInput").ap()
    xbf_d = nc.dram_tensor("xbf", [C, L], BF16, kind="ExternalInput").ap()
    mt8_d = nc.dram_tensor("mt8", [C, C], F8, kind="ExternalInput").ap()
    wvu8_d = nc.dram_tensor("wvu8", [C, 272], F8, kind="ExternalInput").ap()
    bv_d = nc.dram_tensor("bv", [C, 1], F32, kind="ExternalInput").ap()
    out_d = nc.dram_tensor("out", [C, L], BF16, kind="ExternalOutput").ap()

    with tile.TileContext(nc) as tc:
        with (
            tc.tile_pool(name="const", bufs=1) as const,
            tc.tile_pool(name="data", bufs=1) as data,
            tc.tile_pool(name="evict", bufs=4) as evict,
        ):
            # ---- constants / warmup fodder ----
            ones_bf = const.tile([P, NB], BF16)
            nc.vector.memset(ones_bf[:], 1.0)
            ones8 = const.tile([P, 2, 16], F8)
            nc.gpsimd.memset(ones8[:], 1.0)
            tiny = const.tile([P, 2, 16], F32)

            x8 = data.tile([P, 2, L], F8, tag="x8", name="x8")
            xbf = [data.tile([P, L], BF16, tag=f"xbf{c}", name=f"xbf{c}")
                   for c in range(2)]
            mt8 = const.tile([P, 2, C], F8, tag="mt8")
            wvu8 = const.tile([P, 2, 272], F8, tag="wvu8")
            bv_sb = const.tile([P, 2, 1], F32, tag="bv")

            # first l-slice of x8 on several queues, then weights, then rest
            def x8_dma(c0, c1, eng):
                cols = slice(c0, c1)
                eng.dma_start(out=x8[:, :, cols],
                              in_=x8_d[:, cols].rearrange("(j p) l -> p j l",
                                                          p=P))

            x8_dma(0, 768, nc.sync)
            x8_dma(768, 1536, nc.scalar)
            x8_dma(1536, 2048, nc.gpsimd)
            nc.sync.dma_start(out=mt8[:],
                              in_=mt8_d.rearrange("(j p) o -> p j o", p=P))
            nc.scalar.dma_start(out=wvu8[:],
                                in_=wvu8_d.rearrange("(j p) o -> p j o", p=P))
            nc.scalar.dma_start(out=bv_sb[:],
                                in_=bv_d.rearrange("(j p) o -> p j o", p=P))

            w8 = data.tile([P, 2, L], F8, tag="w8", name="w8")
            vT_bf = data.tile([P, NMC, C], BF16, tag="vT")
            pT_bf = data.tile([P, NMC, L], BF16, tag="pT")
            b_act = data.tile([P, NMC, 1], F32, tag="b_act")
            bv_late = data.tile([P, 2, 1], F32, tag="bv_late")
            dacc = data.tile([P, L], BF16, tag="dacc")
            recip = data.tile([P, L], F32, tag="recip")
            xr = [data.tile([P, L], BF16, tag=f"xr{c}", name=f"xr{c}")
                  for c in range(2)]

            # ---- phase 1: PE warmup + w projection ----
            with tc.tile_pool(name="psA", bufs=1, space=bass.MemorySpace.PSUM) as psA:
                # warm the activation tables (one-time ~2.7us DMAs) and the PE
                # HAM clock-gate while x streams in
                warm = psA.tile([P, HALF], F32, tag="wp", name="warm", bufs=2)
                nc.vector.memset(tiny[:, 0, :], 1.0)
                nc.scalar.activation(out=tiny[:, 1, :], in_=tiny[:, 0, :],
                                     func=mybir.ActivationFunctionType.Exp,
                                     scale=1.0)
                nc.vector.reciprocal_approx_fast(out=tiny[:, 1, :],
                                                 in_=tiny[:, 0, :])
                for i in range(WARMUP_MMS):
                    nc.tensor.matmul(warm[:, 0:NB], ones_bf[:, 0:P],
                                     ones_bf[:], start=True, stop=True)
                nc.tensor.matmul(warm[0:16, 0:16], ones8[:], ones8[:],
                                 start=True, stop=True, perf_mode=DR)

                # w = kappa M x  (kappa baked into mt8 on host); one DoubleRow
                # matmul contracts the full 256 channels
                for h in range(2):
                    hcols = slice(h * HALF, (h + 1) * HALF)
                    for oc in range(2):
                        wp = psA.tile([P, HALF], F32, tag="wp", name="wp",
                                      bufs=2)
                        for ln in range(2):
                            c0 = h * HALF + ln * NB
                            nc.tensor.matmul(
                                wp[:, ln * NB:(ln + 1) * NB],
                                mt8[:, :, oc * P:(oc + 1) * P],
                                x8[:, :, c0:c0 + NB],
                                start=True, stop=True, perf_mode=DR)
                        nc.vector.tensor_copy(out=w8[:, oc, hcols], in_=wp[:])

            # xbf for the residual - only needed by the epilogue; these queues
            # are idle during the scores phase
            nc.sync.dma_start(out=xbf[0][:], in_=xbf_d[0:P, :])
            nc.gpsimd.dma_start(out=xbf[1][:], in_=xbf_d[P:C, :])

            # ---- phase 2: v-projection + scores + exp + denom + ctx-left ----
            with tc.tile_pool(name="psCL", bufs=1,
                              space=bass.MemorySpace.PSUM) as psCL:
                ctxL = {(qt, cc): psCL.tile([P, NB], F32, tag=f"cl{qt}{cc}",
                                            name=f"cl{qt}{cc}", bufs=1)
                        for qt in range(2) for cc in range(2)}

                with tc.tile_pool(name="psB", bufs=1,
                                  space=bass.MemorySpace.PSUM) as psB:
                    for mc in range(NMC):
                        mrows = slice(mc * P, (mc + 1) * P)
                        # v/ux projection for this key chunk
                        vp = psB.tile([P, 272], F32, tag="vp", name="vp", bufs=1)
                        nc.tensor.matmul(
                            vp[:], x8[:, :, mrows], wvu8[:],
                            start=True, stop=True, perf_mode=DR)
                        nc.vector.tensor_copy(out=vT_bf[:, mc, :],
                                              in_=vp[:, 0:C])
                        nc.vector.tensor_scalar_add(out=b_act[:, mc, :],
                                                    in0=vp[:, C:C + 1],
                                                    scalar1=-SHIFT)
                        # scores + exp, 512 columns at a time
                        for qt in range(4):
                            s = psB.tile([P, NB], F32, tag="s", name="s",
                                         bufs=3)
                            nc.tensor.matmul(
                                s[:], w8[:, :, mrows],
                                x8[:, :, qt * NB:(qt + 1) * NB],
                                start=True, stop=True, perf_mode=DR)
                            nc.scalar.activation(
                                out=pT_bf[:, mc, qt * NB:(qt + 1) * NB],
                                in_=s[:],
                                func=mybir.ActivationFunctionType.Exp,
                                scale=LN2 / 128.0, bias=b_act[:, mc, :])
                        # running denominator (bf16 accumulator on DVE)
                        src = pT_bf[:, mc, :]
                        if mc == 0:
                            nc.vector.tensor_copy(out=dacc[:], in_=src)
                        else:
                            nc.vector.tensor_add(dacc[:], dacc[:], src)
                        # ctx-left accumulates chunk by chunk (bf16)
                        for cc in range(2):
                            for qt in range(2):
                                nc.tensor.matmul(
                                    ctxL[(qt, cc)][:],
                                    vT_bf[:, mc, cc * P:(cc + 1) * P],
                                    pT_bf[:, mc, qt * NB:(qt + 1) * NB],
                                    start=(mc == 0), stop=(mc == NMC - 1))

                # ---- phase 3: denom matmuls + ctx-right + epilogue ----
                with tc.tile_pool(name="psDR", bufs=1,
                                  space=bass.MemorySpace.PSUM) as psDR:
                    def ds_recip(ln):
                        cols = slice(ln * NB, (ln + 1) * NB)
                        ds = psDR.tile([P, NB], F32, tag="ds", name="ds", bufs=2)
                        nc.tensor.matmul(ds[:], ones_bf[:, 0:P], dacc[:, cols],
                                         start=True, stop=True)
                        nc.vector.reciprocal_approx_fast(out=recip[:, cols],
                                                         in_=ds[:])
                        return ds

                    def ctx_mms(ct, qt, cc):
                        for mc in range(NMC):
                            nc.tensor.matmul(
                                ct[:],
                                vT_bf[:, mc, cc * P:(cc + 1) * P],
                                pT_bf[:, mc, qt * NB:(qt + 1) * NB],
                                start=(mc == 0), stop=(mc == NMC - 1))

                    def ct_evict(ct, qt, cc, nsub, qpick):
                        rows = slice(cc * P, (cc + 1) * P)
                        sub = NB // nsub
                        for si in range(nsub):
                            c0 = qt * NB + si * sub
                            cols = slice(c0, c0 + sub)
                            pcols = slice(si * sub, (si + 1) * sub)
                            t = evict.tile([P, sub], F32, tag="t", name="t")
                            nc.vector.tensor_mul(t[:], ct[:, pcols],
                                                 recip[:, cols])
                            o = evict.tile([P, sub], BF16, tag="o", name="o")
                            nc.gpsimd.tensor_add(o[:], t[:], xr[cc][:, cols])
                            deng = nc.sync if (qpick + si) % 2 == 0 else nc.scalar
                            deng.dma_start(out=out_d[rows, cols], in_=o[:])

                    # ds matmuls interleave between ctx-right tiles so the PE
                    # never head-blocks on the reciprocal chain
                    ds0 = ds_recip(0)
                    ds_recip(1)
                    # residual prep, pinned behind the denominator so the
                    # scheduler cannot hoist it into the scores-phase DVE queue
                    nc.vector.tensor_scalar(out=bv_late[:], in0=bv_sb[:],
                                            scalar1=ds0[:, 0:1],
                                            scalar2=ds0[:, 0:1],
                                            op0=mybir.AluOpType.add,
                                            op1=mybir.AluOpType.subtract)
                    for cc in range(2):
                        nc.vector.tensor_scalar_add(out=xr[cc][:],
                                                    in0=xbf[cc][:],
                                                    scalar1=bv_late[:, cc, :])
                    ctxR = {}
                    for k, (qt, cc) in enumerate(((2, 0), (2, 1), (3, 0), (3, 1))):
                        ct = psDR.tile([P, NB], F32, tag="cr", name="cr", bufs=2)
                        ctxR[(qt, cc)] = ct
                        ctx_mms(ct, qt, cc)
                        if k == 0:
                            ds_recip(2)
                            ds_recip(3)
                            ct_evict(ctxL[(0, 0)], 0, 0, 1, 0)
                            ct_evict(ctxL[(0, 1)], 0, 1, 1, 1)
                        elif k == 1:
                            ct_evict(ctxL[(1, 0)], 1, 0, 1, 0)
                            ct_evict(ctxL[(1, 1)], 1, 1, 1, 1)
                            ct_evict(ctxR[(2, 0)], 2, 0, 1, 0)
                        elif k == 2:
                            ct_evict(ctxR[(2, 1)], 2, 1, 2, 1)
                    ct_evict(ctxR[(3, 0)], 3, 0, 2, 0)
                    ct_evict(ctxR[(3, 1)], 3, 1, 4, 1)

    nc.compile()
    return nc


def get_compiled():
    global _COMPILED
    if _COMPILED is None:
        _COMPILED = build_nc()
    return _COMPILED


def make_in_maps(inputs):
    f8 = ml_dtypes.float8_e4m3
    x = np.ascontiguousarray(np.asarray(inputs["x"], dtype=np.float32))
    Wq = np.asarray(inputs["Wq"], np.float32)
    Wk = np.asarray(inputs["Wk"], np.float32)
    Wv = np.asarray(inputs["Wv"], np.float32)
    bq = np.asarray(inputs["bq"], np.float32)
    M = Wq.T @ Wk                               # scores_raw = x^T M x
    u = SCALE * (Wk.T @ bq)                     # per-key score bias u.x
    wvu = np.zeros((C, 272), np.float32)
    wvu[:, 0:C] = Wv.T
    wvu[:, C] = u
    shared = {
        "mt8": np.ascontiguousarray(KAPPA * M.T).astype(f8),
        "wvu8": wvu.astype(f8),
        "bv": np.asarray(inputs["bv"], np.float32).reshape(C, 1),
    }
    return [{"x8": x[i].astype(f8), "xbf": x[i].astype(ml_dtypes.bfloat16),
             **shared} for i in range(B)]


def run(inputs, trace=False, **kwargs):
    nc = get_compiled()
    res = run_bass_kernel_spmd(nc, make_in_maps(inputs),
                               core_ids=list(range(B)), trace=trace, **kwargs)
    out = np.stack([res.results[i]["out"] for i in range(B)], axis=0)
    return out.astype(np.float32), res


def kernel(**inputs):
    out, _ = run(inputs)
    return out
